# revision 10
# baseline (speedup 1.0000x reference)
"""Trainium2 Bass kernel for dual-input multi-head attention.

Computes, for each of two independent inputs x, y of shape [8, 1024, 768]:
    qkv = inp @ w_qkv.T ; split into 12 heads of 64
    attn = softmax(q k^T / sqrt(64)) v
    out  = attn @ w_proj.T + b_proj
Sharded data-parallel over the batch dim: core i handles batch i of x AND
batch i of y (16 batch-units over 8 cores = 2 per core).

Per-core design (v2 — flat exp pipeline):
  - Host pre-transposes and casts to bf16: inpT [C, N], w_qkvT [C, 3C],
    w_projT [C, C]. All matmuls run in bf16 (1 cycle/row on the PE) with
    fp32 PSUM accumulation.
  - QKV matmuls produce q,k TRANSPOSED ([head_dim, N] per head, as 12
    j-tiles of [128, N]) and v in natural [N, head_dim] layout with a
    column of ones appended, so the P@V matmul also emits the softmax
    denominator as a 65th output row for free.
  - Scores are computed transposed (pT[m, n] = k_m . q_n, contraction=64,
    two heads packed in PE row-quadrants so they run concurrently), exp on
    ScalarE straight out of PSUM (scale folded in; no max-subtraction —
    scores are O(+-15) so exp stays in fp32 range).
  - THE CLOCK: ScalarE's exp stream (192 x ~1.1us) paces the attention
    phases. All 192 (input, pair, half, key-tile) iterations form ONE flat
    pipeline with a global one-iteration score lookahead, so the exp
    stream never bubbles at pair/half/input boundaries (the v1 kernel lost
    ~90us to those bubbles).
  - PSUM is partitioned into dedicated pools (scores 2x2 banks, P@V 2x1,
    filler chains 2x1) so P@V accumulator allocation at section boundaries
    can never block the filler matmul chains.
  - Cross-phase software pipelining: ALL non-attention matmul work (QKV of
    both inputs beyond the prologue, projection of x) drains from a single
    continuous filler queue inside the exp-paced attention windows; only
    the prologue (QKV(x) needed by pair 0) and proj(y) run bare.
  - Normalization: denominator rows gathered at partitions 0/32/64/96,
    one reciprocal_approx_fast (18-bit — way below bf16 noise), staged to
    partition-0 rows, partition-broadcast AND multiplied on the otherwise
    idle GpSimd engine, keeping DVE free for PSUM->SBUF copy-outs.
"""

from collections import deque

import numpy as np

import concourse.bacc as bacc
import concourse.mybir as mybir
import concourse.tile as tile
from concourse import bass_utils

B, N, C, H, HD = 8, 1024, 768, 12, 64
NT = N // 128  # 8 token tiles
CT = C // 128  # 6 contraction chunks
SCALE = HD ** -0.5
F32 = mybir.dt.float32
BF16 = mybir.dt.bfloat16
AF = mybir.ActivationFunctionType
ALU = mybir.AluOpType
N_CORES = 8


def build_program():
    nc = bacc.Bacc("TRN2", target_bir_lowering=False, debug=False)
    inp_dram = [
        nc.dram_tensor("xT", [C, N], BF16, kind="ExternalInput"),
        nc.dram_tensor("yT", [C, N], BF16, kind="ExternalInput"),
    ]
    wqT = nc.dram_tensor("wqT", [C, 3 * C], BF16, kind="ExternalInput")
    wpT = nc.dram_tensor("wpT", [C, C], BF16, kind="ExternalInput")
    bp = nc.dram_tensor("bp", [1, C], F32, kind="ExternalInput")
    out_dram = [
        nc.dram_tensor("out_x", [N, C], F32, kind="ExternalOutput"),
        nc.dram_tensor("out_y", [N, C], F32, kind="ExternalOutput"),
    ]

    with tile.TileContext(nc) as tc:
        with (
            tc.tile_pool(name="pers", bufs=1) as pers,
            tc.tile_pool(name="dbl", bufs=2) as dbl,
            tc.tile_pool(name="pexp", bufs=4) as pep,
            tc.tile_pool(name="pvu", bufs=8) as pvup,
            tc.tile_pool(name="small", bufs=1) as smp,
            tc.tile_pool(name="rbsb", bufs=2) as rbsbp,
            tc.tile_pool(name="outp", bufs=2) as outp,
            tc.tile_pool(name="scps", bufs=2, space="PSUM") as scp,
            tc.tile_pool(name="pvps", bufs=2, space="PSUM") as pvp,
            tc.tile_pool(name="flps", bufs=2, space="PSUM") as flp,
        ):
            # startup-critical DMAs first. The prologue (QKV(x) j-tiles 0
            # and 6 plus the g=0 v chunks) needs x fully, wq cols [0:896]
            # (q j-tiles + k j-tile 6) and wq cols [1536:2304] (v). Order
            # the transfers so the first matmul chain can start as early
            # as possible and the prologue never waits on cold columns.
            wq_sb = pers.tile([128, CT, 3 * C], BF16, name="wq_sb")
            inp_sb = {
                0: dbl.tile([128, CT, N], BF16, name="inp_sb", tag="inp"),
                1: dbl.tile([128, CT, N], BF16, name="inp_sb2", tag="inp"),
            }
            for c in range(CT):
                nc.sync.dma_start(
                    inp_sb[0][:, c, :], inp_dram[0][c * 128 : (c + 1) * 128, :]
                )
                nc.sync.dma_start(
                    wq_sb[:, c, 0:896], wqT[c * 128 : (c + 1) * 128, 0:896]
                )
            for c in range(CT):
                nc.sync.dma_start(
                    wq_sb[:, c, 1536:2304], wqT[c * 128 : (c + 1) * 128, 1536:2304]
                )
            for c in range(CT):
                nc.sync.dma_start(
                    wq_sb[:, c, 896:1536], wqT[c * 128 : (c + 1) * 128, 896:1536]
                )
            for c in range(CT):
                nc.sync.dma_start(
                    inp_sb[1][:, c, :], inp_dram[1][c * 128 : (c + 1) * 128, :]
                )
            wp_sb = pers.tile([128, CT, C], BF16, name="wp_sb")
            for c in range(CT):
                nc.sync.dma_start(wp_sb[:, c, :], wpT[c * 128 : (c + 1) * 128, :])
            b_row = pers.tile([1, C], F32, name="b_row")
            nc.sync.dma_start(b_row[:], bp[:, :])
            bias_sb = pers.tile([128, C], F32, name="bias_sb")
            nc.gpsimd.partition_broadcast(bias_sb[:], b_row[:1, :])

            qkT_sb, v_sb, attnT_sb = {}, {}, {}
            for idx in range(2):
                # q,k transposed: j-tiles 0..5 = q (2 heads/tile), 6..11 = k
                qkT_sb[idx] = dbl.tile([128, H, N], BF16, name="qkT_sb", tag="qkT")
                # v per (token-tile, head): 64 cols of v then one col of ones
                v_sb[idx] = dbl.tile([128, NT, H, HD + 1], BF16, name="v_sb", tag="v")
                nc.vector.memset(v_sb[idx][:, :, :, HD : HD + 1], 1.0)
                # attention output, transposed [C, N] as 6 chunks of 128
                attnT_sb[idx] = dbl.tile([128, CT, N], BF16, name="attnT_sb", tag="attnT")

            def emit_qkT(idx, jt, copy_engine, gs=(0, 1)):
                for _ in gen_qkT(idx, jt, copy_engine, gs):
                    pass

            def gen_qkT(idx, jt, copy_engine, gs=(0, 1)):
                # qkvT[j, n] = sum_c w_qkvT[c, j] inpT[c, n]
                for g in gs:
                    ps = flp.tile([128, 512], F32, name="ps_qk", tag="fl")
                    for c in range(CT):
                        nc.tensor.matmul(
                            ps[:],
                            wq_sb[:, c, jt * 128 : (jt + 1) * 128],
                            inp_sb[idx][:, c, g * 512 : (g + 1) * 512],
                            start=(c == 0),
                            stop=(c == CT - 1),
                        )
                        yield
                    dst = qkT_sb[idx][:, jt, g * 512 : (g + 1) * 512]
                    if copy_engine == "act":
                        nc.scalar.copy(dst, ps[:])
                    elif copy_engine == "pool":
                        nc.gpsimd.tensor_copy(dst, ps[:])
                    else:
                        nc.vector.tensor_copy(dst, ps[:])

            def emit_v(idx, nt, g, copy_engine):
                for _ in gen_v(idx, nt, g, copy_engine):
                    pass

            def gen_v(idx, nt, g, copy_engine):
                # v[n, j] = sum_c inpT[c, n] w_qkvT[c, 2C + j]
                w = 512 if g == 0 else 256
                ps = flp.tile([128, 512], F32, name="ps_v", tag="fl")
                for c in range(CT):
                    nc.tensor.matmul(
                        ps[:, :w],
                        inp_sb[idx][:, c, nt * 128 : (nt + 1) * 128],
                        wq_sb[:, c, 2 * C + g * 512 : 2 * C + g * 512 + w],
                        start=(c == 0),
                        stop=(c == CT - 1),
                    )
                    yield
                hview = ps[:, :w].rearrange("p (h d) -> p h d", d=HD)
                dst = v_sb[idx][:, nt, g * 8 : g * 8 + w // HD, 0:HD]
                if copy_engine == "act":
                    nc.scalar.copy(dst, hview)
                elif copy_engine == "pool":
                    nc.gpsimd.tensor_copy(dst, hview)
                else:
                    nc.vector.tensor_copy(dst, hview)

            def emit_proj(idx, nt):
                for _ in gen_proj(idx, nt):
                    pass

            def gen_proj(idx, nt):
                # out[n, :] = attnT[:, n].T @ wpT + bias, in two free-halves
                # with sequential (not simultaneously-held) PSUM tiles so a
                # 2-buf filler pool never deadlocks.
                out_sb = outp.tile([128, C], F32, name="out_sb", tag="outsb")
                p1 = flp.tile([128, 512], F32, name="p1", tag="fl")
                for c in range(CT):
                    nc.tensor.matmul(
                        p1[:],
                        attnT_sb[idx][:, c, nt * 128 : (nt + 1) * 128],
                        wp_sb[:, c, 0:512],
                        start=(c == 0),
                        stop=(c == CT - 1),
                    )
                    yield
                nc.vector.tensor_tensor(
                    out_sb[:, 0:512], p1[:], bias_sb[:, 0:512], op=ALU.add
                )
                p2 = flp.tile([128, 512], F32, name="p2", tag="fl")
                for c in range(CT):
                    nc.tensor.matmul(
                        p2[:, :256],
                        attnT_sb[idx][:, c, nt * 128 : (nt + 1) * 128],
                        wp_sb[:, c, 512:768],
                        start=(c == 0),
                        stop=(c == CT - 1),
                    )
                    yield
                nc.vector.tensor_tensor(
                    out_sb[:, 512:768], p2[:, :256], bias_sb[:, 512:768], op=ALU.add
                )
                nc.sync.dma_start(out_dram[idx][nt * 128 : (nt + 1) * 128, :], out_sb[:])

            fillers = deque()  # generators yielding once per PE matmul

            def drain_mm(k):
                # advance filler work by k PE matmuls
                while k > 0 and fillers:
                    try:
                        next(fillers[0])
                        k -= 1
                    except StopIteration:
                        fillers.popleft()

            def drain_all():
                while fillers:
                    try:
                        next(fillers[0])
                    except StopIteration:
                        fillers.popleft()

            def emit_norm(idx, t, pvu):
                # batched softmax denominators: gather at partitions 0/32/64/96,
                # one approx reciprocal (18-bit, ~5x faster than exact; way
                # below bf16 noise), stage each row back to partition 0
                # (GpSimd partition_broadcast only reads partition 0 on HW),
                # then broadcast AND multiply on the idle GpSimd engine so
                # DVE stays free for copy-outs.
                keys = list(pvu)
                sums4 = smp.tile([128, 512], F32, name="sums4", tag="sums")
                nc.vector.memset(sums4[:], 1.0)
                for r, k in enumerate(keys):
                    nc.vector.tensor_copy(
                        sums4[32 * r : 32 * r + 1, :], pvu[k][HD : HD + 1, :]
                    )
                recip4 = smp.tile([128, 512], F32, name="recip4", tag="recip")
                nc.vector.reciprocal_approx_fast(recip4[0:97, :], sums4[0:97, :])
                for r, (ab, g) in enumerate(keys):
                    h = 2 * t + ab
                    hc, pb = h // 2, (h % 2) * 64
                    if r == 0:
                        stage = recip4
                    else:
                        stage = smp.tile([1, 512], F32, name=f"st{r}", tag=f"st{r}")
                        nc.vector.tensor_copy(stage[0:1, :], recip4[32 * r : 32 * r + 1, :])
                    rb_sb = rbsbp.tile([64, 512], F32, name="rb_sb", tag="rb")
                    nc.gpsimd.partition_broadcast(rb_sb[:], stage[0:1, :])
                    nc.vector.tensor_tensor(
                        attnT_sb[idx][pb : pb + 64, hc, g * 512 : (g + 1) * 512],
                        pvu[(ab, g)][0:HD, :],
                        rb_sb[:],
                        op=ALU.mult,
                    )

            # ---- flat exp-paced pipeline over all attention work ----
            # sections: (input, pair, n-half); 8 key-tile iterations each.
            sections = [
                (idx, t, g) for idx in range(2) for t in range(H // 2) for g in range(2)
            ]
            NSEC = len(sections)

            def sc_exp(s, mt):
                # one tile holds this n-half's scores for BOTH heads of the
                # section's pair; the two contraction-64 score matmuls sit in
                # disjoint PE row quadrants and run concurrently. The whole
                # scores->exp chain is THE pipeline clock, so it gets top
                # scheduler priority: the list scheduler must never split the
                # quadrant pair or park filler/P@V work ahead of it.
                idx, t, g = sections[s]
                sc = scp.tile([128, 2, 512], F32, name="sc", tag="sc")
                with tc.high_priority(offset=25):
                    for ab in range(2):
                        pb = ab * 64
                        nc.tensor.matmul(
                            sc[:, ab, :],
                            qkT_sb[idx][pb : pb + 64, 6 + t, mt * 128 : (mt + 1) * 128],
                            qkT_sb[idx][pb : pb + 64, t, g * 512 : (g + 1) * 512],
                            start=True,
                            stop=True,
                            tile_position=(pb, 0),
                        )
                    pe = pep.tile([128, N], BF16, name="pe", tag="pexp")
                    nc.scalar.activation(
                        pe[:],
                        sc[:].rearrange("p a b -> p (a b)"),
                        AF.Exp,
                        scale=SCALE,
                    )
                return pe

            # prologue: only what the first attention section needs -- q/k
            # j-tiles 0 and 6 of x plus the 512-col v chunks (heads 0-7).
            # Copies on DVE (ScalarE must stay exp-only once the stream
            # starts; everything is idle here anyway).
            emit_qkT(0, 0, "dve")
            emit_qkT(0, 6, "dve")
            for nt in range(NT):
                emit_v(0, nt, 0, "dve")

            # continuous filler queue, ordered by when results are needed:
            # x pairs 1-5 j-tiles, x v tail, y pair-0 j-tiles, y v g0,
            # y remaining j-tiles, y v tail. proj(x) is appended mid-flight
            # once attnT(x) is fully normalized; proj(y) runs bare at the
            # end. (GpSimd cannot read PSUM on HW, so all PSUM->SBUF
            # copy-outs ride DVE; GpSimd handles the SBUF-only broadcasts.)
            for t in range(1, H // 2):
                fillers.append(gen_qkT(0, t, "dve"))
                fillers.append(gen_qkT(0, 6 + t, "dve"))
            for nt in range(NT):
                fillers.append(gen_v(0, nt, 1, "dve"))
            fillers.append(gen_qkT(1, 0, "dve"))
            fillers.append(gen_qkT(1, 6, "dve"))
            for nt in range(NT):
                fillers.append(gen_v(1, nt, 0, "dve"))
            for t in range(1, H // 2):
                fillers.append(gen_qkT(1, t, "dve"))
                fillers.append(gen_qkT(1, 6 + t, "dve"))
            for nt in range(NT):
                fillers.append(gen_v(1, nt, 1, "dve"))

            pend_norm = None
            pair_pvu = {}
            carry = 0  # drains deferred past the next iteration's scores
            pe_next = sc_exp(0, 0)
            for it in range(NSEC * NT):
                s, mt = divmod(it, NT)
                idx, t, g = sections[s]
                pe_cur = pe_next
                if it + 1 < NSEC * NT:
                    pe_next = sc_exp(*divmod(it + 1, NT))
                # extra section-end drains land here, AFTER the next scores
                # are already in the PE queue, so the exp stream (the clock)
                # never waits behind a filler burst.
                drain_mm(carry)
                carry = 0
                if mt == 0:
                    if g == 1 and pend_norm is not None:
                        # previous pair's normalization launches mid-pair so
                        # pair boundaries only carry the pvu copies
                        emit_norm(*pend_norm)
                        pend_norm = None
                    if s == 13:
                        # attnT(x) fully normalized (norm of x pair 5 just
                        # emitted at s==13's g==1 slot... it was emitted at
                        # the s==13 norm slot above); queue proj(x).
                        for nt in range(NT):
                            fillers.append(gen_proj(0, nt))
                    # allocate this section's P@V accumulators
                    pv = {
                        ab: pvp.tile([HD + 1, 512], F32, name="pv", tag="pv")
                        for ab in range(2)
                    }
                # fillers go before the P@V matmuls: P@V waits on the exp
                # semaphore, and the PE is in-order, so always-ready filler
                # work must sit ahead of the potentially-waiting instruction.
                drain_mm(2)
                for ab in range(2):
                    nc.tensor.matmul(
                        pv[ab],
                        v_sb[idx][:, mt, 2 * t + ab, :],
                        pe_cur[:, ab * 512 : (ab + 1) * 512],
                        start=(mt == 0),
                        stop=(mt == NT - 1),
                    )
                if mt == NT - 1:
                    last = s == NSEC - 1
                    for ab in range(2):
                        u = pvup.tile([HD + 1, 512], F32, name="pvu", tag="pvu")
                        # final section: copy on ScalarE (idle once the exp
                        # stream ends) so the DVE queue doesn't delay the
                        # last PSUM release. High priority: the next
                        # section's P@V accumulators wait on these.
                        if last:
                            nc.scalar.copy(u[:], pv[ab][:])
                        else:
                            with tc.high_priority(offset=10):
                                nc.vector.tensor_copy(u[:], pv[ab][:])
                        pair_pvu[(ab, g)] = u
                    if g == 1:
                        pend_norm = (idx, t, dict(pair_pvu))
                        pair_pvu = {}
                    carry = 4

            emit_norm(*pend_norm)
            drain_all()
            for nt in range(NT):
                emit_proj(1, nt)

    nc.compile()
    return nc


_PROGRAM = None


def _get_program():
    global _PROGRAM
    if _PROGRAM is None:
        _PROGRAM = build_program()
    return _PROGRAM


def make_in_maps(x, y, w_qkv, w_proj, b_proj):
    import ml_dtypes

    bf = ml_dtypes.bfloat16
    x = np.asarray(x, np.float32)
    y = np.asarray(y, np.float32)
    xT = np.ascontiguousarray(x.transpose(0, 2, 1)).astype(bf)
    yT = np.ascontiguousarray(y.transpose(0, 2, 1)).astype(bf)
    wqT = np.ascontiguousarray(np.asarray(w_qkv, np.float32).T).astype(bf)
    wpT = np.ascontiguousarray(np.asarray(w_proj, np.float32).T).astype(bf)
    bp = np.ascontiguousarray(np.asarray(b_proj, np.float32).reshape(1, C))
    return [
        {"xT": xT[i], "yT": yT[i], "wqT": wqT, "wpT": wpT, "bp": bp}
        for i in range(N_CORES)
    ]


def kernel(x, y, w_qkv, w_proj, b_proj):
    nc = _get_program()
    in_maps = make_in_maps(x, y, w_qkv, w_proj, b_proj)
    res = bass_utils.run_bass_kernel_spmd(nc, in_maps, core_ids=list(range(N_CORES)))
    xo = np.stack([np.asarray(res.results[i]["out_x"]) for i in range(N_CORES)])
    yo = np.stack([np.asarray(res.results[i]["out_y"]) for i in range(N_CORES)])
    return (xo, yo)


# revision 11
# speedup vs baseline: 1.2100x; 1.2100x over previous
"""Trainium2 Bass kernel for dual-input multi-head attention.

Computes, for each of two independent inputs x, y of shape [8, 1024, 768]:
    qkv = inp @ w_qkv.T ; split into 12 heads of 64
    attn = softmax(q k^T / sqrt(64)) v
    out  = attn @ w_proj.T + b_proj
Sharded data-parallel over the batch dim: core i handles batch i of x AND
batch i of y (16 batch-units over 8 cores = 2 per core).

Per-core design (v2 — flat exp pipeline):
  - Host pre-transposes and casts to bf16: inpT [C, N], w_qkvT [C, 3C],
    w_projT [C, C]. All matmuls run in bf16 (1 cycle/row on the PE) with
    fp32 PSUM accumulation.
  - QKV matmuls produce q,k TRANSPOSED ([head_dim, N] per head, as 12
    j-tiles of [128, N]) and v in natural [N, head_dim] layout with a
    column of ones appended, so the P@V matmul also emits the softmax
    denominator as a 65th output row for free.
  - Scores are computed transposed (pT[m, n] = k_m . q_n, contraction=64,
    two heads packed in PE row-quadrants so they run concurrently), exp on
    ScalarE straight out of PSUM (scale folded in; no max-subtraction —
    scores are O(+-15) so exp stays in fp32 range).
  - THE CLOCK: ScalarE's exp stream (192 x ~1.1us) paces the attention
    phases. All 192 (input, pair, half, key-tile) iterations form ONE flat
    pipeline with a global one-iteration score lookahead, so the exp
    stream never bubbles at pair/half/input boundaries (the v1 kernel lost
    ~90us to those bubbles).
  - PSUM is partitioned into dedicated pools (scores 2x2 banks, P@V 2x1,
    filler chains 2x1) so P@V accumulator allocation at section boundaries
    can never block the filler matmul chains.
  - Cross-phase software pipelining: ALL non-attention matmul work (QKV of
    both inputs beyond the prologue, projection of x) drains from a single
    continuous filler queue inside the exp-paced attention windows; only
    the prologue (QKV(x) needed by pair 0) and proj(y) run bare.
  - Normalization: denominator rows gathered at partitions 0/32/64/96,
    one reciprocal_approx_fast (18-bit — way below bf16 noise), staged to
    partition-0 rows, partition-broadcast AND multiplied on the otherwise
    idle GpSimd engine, keeping DVE free for PSUM->SBUF copy-outs.
"""

from collections import deque

import numpy as np

import concourse.bacc as bacc
import concourse.mybir as mybir
import concourse.tile as tile
from concourse import bass_utils

B, N, C, H, HD = 8, 1024, 768, 12, 64
NT = N // 128  # 8 token tiles
CT = C // 128  # 6 contraction chunks
SCALE = HD ** -0.5
F32 = mybir.dt.float32
BF16 = mybir.dt.bfloat16
AF = mybir.ActivationFunctionType
ALU = mybir.AluOpType
N_CORES = 8


def build_program():
    nc = bacc.Bacc("TRN2", target_bir_lowering=False, debug=False)
    inp_dram = [
        nc.dram_tensor("xT", [C, N], BF16, kind="ExternalInput"),
        nc.dram_tensor("yT", [C, N], BF16, kind="ExternalInput"),
    ]
    wqT = nc.dram_tensor("wqT", [C, 3 * C], BF16, kind="ExternalInput")
    wpT = nc.dram_tensor("wpT", [C, C], BF16, kind="ExternalInput")
    bp = nc.dram_tensor("bp", [1, C], F32, kind="ExternalInput")
    out_dram = [
        nc.dram_tensor("out_x", [N, C], F32, kind="ExternalOutput"),
        nc.dram_tensor("out_y", [N, C], F32, kind="ExternalOutput"),
    ]

    with tile.TileContext(nc) as tc:
        with (
            tc.tile_pool(name="pers", bufs=1) as pers,
            tc.tile_pool(name="dbl", bufs=2) as dbl,
            tc.tile_pool(name="pexp", bufs=4) as pep,
            tc.tile_pool(name="pvu", bufs=8) as pvup,
            tc.tile_pool(name="small", bufs=1) as smp,
            tc.tile_pool(name="rbsb", bufs=2) as rbsbp,
            tc.tile_pool(name="outp", bufs=2) as outp,
            tc.tile_pool(name="scps", bufs=2, space="PSUM") as scp,
            tc.tile_pool(name="pvps", bufs=2, space="PSUM") as pvp,
            tc.tile_pool(name="flps", bufs=2, space="PSUM") as flp,
        ):
            # startup-critical DMAs first. The prologue (QKV(x) j-tiles 0
            # and 6 plus the g=0 v chunks) needs x fully, wq cols [0:896]
            # (q j-tiles + k j-tile 6) and wq cols [1536:2304] (v). Order
            # the transfers so the first matmul chain can start as early
            # as possible and the prologue never waits on cold columns.
            wq_sb = pers.tile([128, CT, 3 * C], BF16, name="wq_sb")
            inp_sb = {
                0: dbl.tile([128, CT, N], BF16, name="inp_sb", tag="inp"),
                1: dbl.tile([128, CT, N], BF16, name="inp_sb2", tag="inp"),
            }
            for c in range(CT):
                nc.sync.dma_start(
                    inp_sb[0][:, c, :], inp_dram[0][c * 128 : (c + 1) * 128, :]
                )
                nc.sync.dma_start(
                    wq_sb[:, c, 0:896], wqT[c * 128 : (c + 1) * 128, 0:896]
                )
            for c in range(CT):
                nc.sync.dma_start(
                    wq_sb[:, c, 1536:2304], wqT[c * 128 : (c + 1) * 128, 1536:2304]
                )
            for c in range(CT):
                nc.sync.dma_start(
                    wq_sb[:, c, 896:1536], wqT[c * 128 : (c + 1) * 128, 896:1536]
                )
            for c in range(CT):
                nc.sync.dma_start(
                    inp_sb[1][:, c, :], inp_dram[1][c * 128 : (c + 1) * 128, :]
                )
            wp_sb = pers.tile([128, CT, C], BF16, name="wp_sb")
            for c in range(CT):
                nc.sync.dma_start(wp_sb[:, c, :], wpT[c * 128 : (c + 1) * 128, :])
            b_row = pers.tile([1, C], F32, name="b_row")
            nc.sync.dma_start(b_row[:], bp[:, :])
            bias_sb = pers.tile([128, C], F32, name="bias_sb")
            nc.gpsimd.partition_broadcast(bias_sb[:], b_row[:1, :])

            qkT_sb, v_sb, attnT_sb = {}, {}, {}
            for idx in range(2):
                # q,k transposed: j-tiles 0..5 = q (2 heads/tile), 6..11 = k
                qkT_sb[idx] = dbl.tile([128, H, N], BF16, name="qkT_sb", tag="qkT")
                # v per (token-tile, head): 64 cols of v then one col of ones
                v_sb[idx] = dbl.tile([128, NT, H, HD + 1], BF16, name="v_sb", tag="v")
                nc.vector.memset(v_sb[idx][:, :, :, HD : HD + 1], 1.0)
                # attention output, transposed [C, N] as 6 chunks of 128
                attnT_sb[idx] = dbl.tile([128, CT, N], BF16, name="attnT_sb", tag="attnT")

            def emit_qkT(idx, jt, copy_engine, gs=(0, 1)):
                for _ in gen_qkT(idx, jt, copy_engine, gs):
                    pass

            def gen_qkT(idx, jt, copy_engine, gs=(0, 1)):
                # qkvT[j, n] = sum_c w_qkvT[c, j] inpT[c, n]
                for g in gs:
                    ps = flp.tile([128, 512], F32, name="ps_qk", tag="fl")
                    for c in range(CT):
                        nc.tensor.matmul(
                            ps[:],
                            wq_sb[:, c, jt * 128 : (jt + 1) * 128],
                            inp_sb[idx][:, c, g * 512 : (g + 1) * 512],
                            start=(c == 0),
                            stop=(c == CT - 1),
                        )
                        yield
                    dst = qkT_sb[idx][:, jt, g * 512 : (g + 1) * 512]
                    if copy_engine == "act":
                        nc.scalar.copy(dst, ps[:])
                    elif copy_engine == "pool":
                        nc.gpsimd.tensor_copy(dst, ps[:])
                    else:
                        nc.vector.tensor_copy(dst, ps[:])

            def emit_v(idx, nt, g, copy_engine):
                for _ in gen_v(idx, nt, g, copy_engine):
                    pass

            def gen_v(idx, nt, g, copy_engine):
                # v[n, j] = sum_c inpT[c, n] w_qkvT[c, 2C + j]
                w = 512 if g == 0 else 256
                ps = flp.tile([128, 512], F32, name="ps_v", tag="fl")
                for c in range(CT):
                    nc.tensor.matmul(
                        ps[:, :w],
                        inp_sb[idx][:, c, nt * 128 : (nt + 1) * 128],
                        wq_sb[:, c, 2 * C + g * 512 : 2 * C + g * 512 + w],
                        start=(c == 0),
                        stop=(c == CT - 1),
                    )
                    yield
                hview = ps[:, :w].rearrange("p (h d) -> p h d", d=HD)
                dst = v_sb[idx][:, nt, g * 8 : g * 8 + w // HD, 0:HD]
                if copy_engine == "act":
                    nc.scalar.copy(dst, hview)
                elif copy_engine == "pool":
                    nc.gpsimd.tensor_copy(dst, hview)
                else:
                    nc.vector.tensor_copy(dst, hview)

            def emit_proj(idx, nt):
                for _ in gen_proj(idx, nt):
                    pass

            def gen_proj(idx, nt):
                # out[n, :] = attnT[:, n].T @ wpT + bias, in two free-halves
                # with sequential (not simultaneously-held) PSUM tiles so a
                # 2-buf filler pool never deadlocks.
                out_sb = outp.tile([128, C], F32, name="out_sb", tag="outsb")
                p1 = flp.tile([128, 512], F32, name="p1", tag="fl")
                for c in range(CT):
                    nc.tensor.matmul(
                        p1[:],
                        attnT_sb[idx][:, c, nt * 128 : (nt + 1) * 128],
                        wp_sb[:, c, 0:512],
                        start=(c == 0),
                        stop=(c == CT - 1),
                    )
                    yield
                nc.vector.tensor_tensor(
                    out_sb[:, 0:512], p1[:], bias_sb[:, 0:512], op=ALU.add
                )
                p2 = flp.tile([128, 512], F32, name="p2", tag="fl")
                for c in range(CT):
                    nc.tensor.matmul(
                        p2[:, :256],
                        attnT_sb[idx][:, c, nt * 128 : (nt + 1) * 128],
                        wp_sb[:, c, 512:768],
                        start=(c == 0),
                        stop=(c == CT - 1),
                    )
                    yield
                nc.vector.tensor_tensor(
                    out_sb[:, 512:768], p2[:, :256], bias_sb[:, 512:768], op=ALU.add
                )
                nc.sync.dma_start(out_dram[idx][nt * 128 : (nt + 1) * 128, :], out_sb[:])

            fillers = deque()  # generators yielding once per PE matmul

            def drain_mm(k):
                # advance filler work by k PE matmuls
                while k > 0 and fillers:
                    try:
                        next(fillers[0])
                        k -= 1
                    except StopIteration:
                        fillers.popleft()

            def drain_all():
                while fillers:
                    try:
                        next(fillers[0])
                    except StopIteration:
                        fillers.popleft()

            def emit_norm(idx, t, pvu):
                # batched softmax denominators: gather at partitions 0/32/64/96,
                # one approx reciprocal (18-bit, ~5x faster than exact; way
                # below bf16 noise), stage each row back to partition 0
                # (GpSimd partition_broadcast only reads partition 0 on HW),
                # then broadcast AND multiply on the idle GpSimd engine so
                # DVE stays free for copy-outs.
                keys = list(pvu)
                sums4 = smp.tile([128, 512], F32, name="sums4", tag="sums")
                nc.vector.memset(sums4[:], 1.0)
                for r, k in enumerate(keys):
                    nc.vector.tensor_copy(
                        sums4[32 * r : 32 * r + 1, :], pvu[k][HD : HD + 1, :]
                    )
                recip4 = smp.tile([128, 512], F32, name="recip4", tag="recip")
                nc.vector.reciprocal_approx_fast(recip4[0:97, :], sums4[0:97, :])
                for r, (ab, g) in enumerate(keys):
                    h = 2 * t + ab
                    hc, pb = h // 2, (h % 2) * 64
                    if r == 0:
                        stage = recip4
                    else:
                        stage = smp.tile([1, 512], F32, name=f"st{r}", tag=f"st{r}")
                        nc.vector.tensor_copy(stage[0:1, :], recip4[32 * r : 32 * r + 1, :])
                    rb_sb = rbsbp.tile([64, 512], F32, name="rb_sb", tag="rb")
                    nc.gpsimd.partition_broadcast(rb_sb[:], stage[0:1, :])
                    nc.vector.tensor_tensor(
                        attnT_sb[idx][pb : pb + 64, hc, g * 512 : (g + 1) * 512],
                        pvu[(ab, g)][0:HD, :],
                        rb_sb[:],
                        op=ALU.mult,
                    )

            # ---- flat exp-paced pipeline over all attention work ----
            # sections: (input, pair, n-half); 8 key-tile iterations each.
            sections = [
                (idx, t, g) for idx in range(2) for t in range(H // 2) for g in range(2)
            ]
            NSEC = len(sections)

            def sc_exp(s, mt):
                # one tile holds this n-half's scores for BOTH heads of the
                # section's pair; the two contraction-64 score matmuls sit in
                # disjoint PE row quadrants and run concurrently. The whole
                # scores->exp chain is THE pipeline clock, so it gets top
                # scheduler priority: the list scheduler must never split the
                # quadrant pair or park filler/P@V work ahead of it.
                idx, t, g = sections[s]
                sc = scp.tile([128, 2, 512], F32, name="sc", tag="sc")
                for ab in range(2):
                    pb = ab * 64
                    nc.tensor.matmul(
                        sc[:, ab, :],
                        qkT_sb[idx][pb : pb + 64, 6 + t, mt * 128 : (mt + 1) * 128],
                        qkT_sb[idx][pb : pb + 64, t, g * 512 : (g + 1) * 512],
                        start=True,
                        stop=True,
                        tile_position=(pb, 0),
                    )
                pe = pep.tile([128, N], BF16, name="pe", tag="pexp")
                nc.scalar.activation(
                    pe[:],
                    sc[:].rearrange("p a b -> p (a b)"),
                    AF.Exp,
                    scale=SCALE,
                )
                return pe

            # prologue: only what the first attention section needs -- q/k
            # j-tiles 0 and 6 of x plus the 512-col v chunks (heads 0-7).
            # Copies on DVE (ScalarE must stay exp-only once the stream
            # starts; everything is idle here anyway).
            emit_qkT(0, 0, "dve")
            emit_qkT(0, 6, "dve")
            for nt in range(NT):
                emit_v(0, nt, 0, "dve")

            # continuous filler queue, ordered by when results are needed:
            # x pairs 1-5 j-tiles, x v tail, y pair-0 j-tiles, y v g0,
            # y remaining j-tiles, y v tail. proj(x) is appended mid-flight
            # once attnT(x) is fully normalized; proj(y) runs bare at the
            # end. (GpSimd cannot read PSUM on HW, so all PSUM->SBUF
            # copy-outs ride DVE; GpSimd handles the SBUF-only broadcasts.)
            for t in range(1, H // 2):
                fillers.append(gen_qkT(0, t, "dve"))
                fillers.append(gen_qkT(0, 6 + t, "dve"))
            for nt in range(NT):
                fillers.append(gen_v(0, nt, 1, "dve"))
            fillers.append(gen_qkT(1, 0, "dve"))
            fillers.append(gen_qkT(1, 6, "dve"))
            for nt in range(NT):
                fillers.append(gen_v(1, nt, 0, "dve"))
            for t in range(1, H // 2):
                fillers.append(gen_qkT(1, t, "dve"))
                fillers.append(gen_qkT(1, 6 + t, "dve"))
            for nt in range(NT):
                fillers.append(gen_v(1, nt, 1, "dve"))

            pend_norm = None
            pair_pvu = {}
            carry = 0  # drains deferred past the next iteration's scores
            TOT = NSEC * NT
            # two-deep score lookahead: exp(n+1) AND exp(n+2) have their
            # score matmuls queued before iteration n's P@V work, so the
            # exp stream bridges the P@V-accumulator reuse stall at every
            # section boundary.
            pe_q = deque([sc_exp(0, 0), sc_exp(0, 1)])
            for it in range(TOT):
                s, mt = divmod(it, NT)
                idx, t, g = sections[s]
                pe_cur = pe_q.popleft()
                if it + 2 < TOT:
                    pe_q.append(sc_exp(*divmod(it + 2, NT)))
                # extra section-end drains land here, AFTER the next scores
                # are already in the PE queue, so the exp stream (the clock)
                # never waits behind a filler burst.
                drain_mm(carry)
                carry = 0
                if mt == 0:
                    if g == 1 and pend_norm is not None:
                        # previous pair's normalization launches mid-pair so
                        # pair boundaries only carry the pvu copies
                        emit_norm(*pend_norm)
                        pend_norm = None
                    if s == 13:
                        # attnT(x) fully normalized (norm of x pair 5 just
                        # emitted at s==13's g==1 slot... it was emitted at
                        # the s==13 norm slot above); queue proj(x).
                        for nt in range(NT):
                            fillers.append(gen_proj(0, nt))
                    # allocate this section's P@V accumulators
                    pv = {
                        ab: pvp.tile([HD + 1, 512], F32, name="pv", tag="pv")
                        for ab in range(2)
                    }
                # fillers go before the P@V matmuls: P@V waits on the exp
                # semaphore, and the PE is in-order, so always-ready filler
                # work must sit ahead of the potentially-waiting instruction.
                drain_mm(2)
                for ab in range(2):
                    nc.tensor.matmul(
                        pv[ab],
                        v_sb[idx][:, mt, 2 * t + ab, :],
                        pe_cur[:, ab * 512 : (ab + 1) * 512],
                        start=(mt == 0),
                        stop=(mt == NT - 1),
                    )
                if mt == NT - 1:
                    last = s == NSEC - 1
                    for ab in range(2):
                        u = pvup.tile([HD + 1, 512], F32, name="pvu", tag="pvu")
                        # final section: copy on ScalarE (idle once the exp
                        # stream ends) so the DVE queue doesn't delay the
                        # last PSUM release. High priority: the next
                        # section's P@V accumulators wait on these.
                        if last:
                            nc.scalar.copy(u[:], pv[ab][:])
                        else:
                            nc.vector.tensor_copy(u[:], pv[ab][:])
                        pair_pvu[(ab, g)] = u
                    if g == 1:
                        pend_norm = (idx, t, dict(pair_pvu))
                        pair_pvu = {}
                    carry = 6

            emit_norm(*pend_norm)
            drain_all()
            for nt in range(NT):
                emit_proj(1, nt)

    nc.compile()
    return nc


_PROGRAM = None


def _get_program():
    global _PROGRAM
    if _PROGRAM is None:
        _PROGRAM = build_program()
    return _PROGRAM


def make_in_maps(x, y, w_qkv, w_proj, b_proj):
    import ml_dtypes

    bf = ml_dtypes.bfloat16
    x = np.asarray(x, np.float32)
    y = np.asarray(y, np.float32)
    xT = np.ascontiguousarray(x.transpose(0, 2, 1)).astype(bf)
    yT = np.ascontiguousarray(y.transpose(0, 2, 1)).astype(bf)
    wqT = np.ascontiguousarray(np.asarray(w_qkv, np.float32).T).astype(bf)
    wpT = np.ascontiguousarray(np.asarray(w_proj, np.float32).T).astype(bf)
    bp = np.ascontiguousarray(np.asarray(b_proj, np.float32).reshape(1, C))
    return [
        {"xT": xT[i], "yT": yT[i], "wqT": wqT, "wpT": wpT, "bp": bp}
        for i in range(N_CORES)
    ]


def kernel(x, y, w_qkv, w_proj, b_proj):
    nc = _get_program()
    in_maps = make_in_maps(x, y, w_qkv, w_proj, b_proj)
    res = bass_utils.run_bass_kernel_spmd(nc, in_maps, core_ids=list(range(N_CORES)))
    xo = np.stack([np.asarray(res.results[i]["out_x"]) for i in range(N_CORES)])
    yo = np.stack([np.asarray(res.results[i]["out_y"]) for i in range(N_CORES)])
    return (xo, yo)
